# revision 8
# baseline (speedup 1.0000x reference)
"""Trainium2 Bass kernel for nn_Cross_AgentAttention.

Data-parallel over batch B=8 across 8 NeuronCores; params replicated.

Per-core algorithm (feature-major (c, n) layout, exploiting that
q = guidmap @ q_w + q_b is rank-1):
  - v = x @ Wv via fp32r matmuls (TF32-grade, full PE rate)
  - agent->kv attention collapses to kw[h,i] = w_h . k_h[i], computed as
    x @ Mkw (folded on host); logits = scale*gbar_a*kw[h,i] + PB, where
    row-constant terms cancel in softmax.
  - query->agent attention collapses to a rank-1 logit map
    lq[i,(h,a)] = g_i * u[(h,a)] (+r) + ABt.
  - depthwise 3x3 conv = 9 shifted diagonal matmuls accumulated into the
    same PSUM tile as the attention-output matmul.
"""
import numpy as np
import ml_dtypes

import concourse.bass as bass
import concourse.bacc as bacc
import concourse.mybir as mybir
from concourse.tile import TileContext
from concourse.bass_utils import run_bass_kernel_spmd

F32 = mybir.dt.float32
F32R = mybir.dt.float32r
BF16 = mybir.dt.bfloat16
BF = ml_dtypes.bfloat16

DIM = 256
HEADS = 8
AGENT = 16
H = W = 64
B = 8
N = H * W                 # 4096
HD = DIM // HEADS         # 32
SCALE = HD ** -0.5
PS = 4
NT = 8                    # n-tiles of 512
NTW = N // NT             # 512
NCH = 32                  # n-chunks of 128
ROWS_PER_NT = NTW // W    # 8 image rows per n-tile

AL = mybir.AluOpType
AF = mybir.ActivationFunctionType


# ----------------------------------------------------------------------------
# host precompute
# ----------------------------------------------------------------------------

def _bilinear_matrix(n_in, n_out):
    U = np.zeros((n_out, n_in), dtype=np.float64)
    s = n_in / n_out
    for o in range(n_out):
        x = (o + 0.5) * s - 0.5
        x0 = int(np.floor(x))
        t = x - x0
        for i, wt in ((x0, 1.0 - t), (x0 + 1, t)):
            ic = min(max(i, 0), n_in - 1)
            U[o, ic] += wt
    return U.astype(np.float32)


def _host_precompute(kv_w, kv_b, q_w, q_b, proj_w, proj_b, dwc_w, dwc_b,
                     an_bias, na_bias, ah_bias, aw_bias, ha_bias, wa_bias):
    c = DIM
    w = q_w[0]
    beta = q_b
    U = _bilinear_matrix(PS, H)

    an_up = np.einsum("yr,harc,xc->hayx", U, an_bias.reshape(HEADS, AGENT, PS, PS), U)
    pb = an_up + ah_bias[0][..., 0][:, :, :, None] + aw_bias[0][:, :, 0, :][:, :, None, :]
    PB = pb.reshape(HEADS * AGENT, N).astype(np.float32)

    na_up = np.einsum("yr,harc,xc->hayx", U, na_bias.reshape(HEADS, AGENT, PS, PS), U)
    ab = na_up.reshape(HEADS, AGENT, N).transpose(0, 2, 1)
    ab = ab + (ha_bias[0] + wa_bias[0]).reshape(HEADS, N, AGENT)
    ABt = ab.transpose(1, 0, 2).reshape(N, HEADS * AGENT).astype(np.float32)

    wk = kv_w[:, :c]
    Mkw = np.stack([(wk[:, h*HD:(h+1)*HD] * w[None, h*HD:(h+1)*HD]).sum(1)
                    for h in range(HEADS)], axis=1)
    Mkb = np.stack([(wk[:, h*HD:(h+1)*HD] * beta[None, h*HD:(h+1)*HD]).sum(1)
                    for h in range(HEADS)], axis=1)
    MM = np.concatenate([Mkw, Mkb], axis=1).astype(np.float32)      # (256, 16)

    hw2 = np.array([(w[h*HD:(h+1)*HD]**2).sum() for h in range(HEADS)], np.float32)
    wb = np.array([(w[h*HD:(h+1)*HD]*beta[h*HD:(h+1)*HD]).sum() for h in range(HEADS)], np.float32)
    bb = np.array([(beta[h*HD:(h+1)*HD]**2).sum() for h in range(HEADS)], np.float32)
    # qrows: [s*hw2 | s*wb | s*bb] repeated per agent -> (1, 384)
    qrows = np.concatenate([np.repeat(SCALE * hw2, AGENT),
                            np.repeat(SCALE * wb, AGENT),
                            np.repeat(SCALE * bb, AGENT)])[None, :].astype(np.float32)

    Wv = kv_w[:, c:].astype(np.float32)                              # (256, 256)
    bv = kv_b[c:].astype(np.float32)

    headmask = np.zeros((HEADS * AGENT, c), np.float32)
    for h in range(HEADS):
        headmask[h*AGENT:(h+1)*AGENT, h*HD:(h+1)*HD] = 1.0

    # EgC: constant rows 8..15 of Eg (selector for the k-beta stream)
    EgC = np.zeros((8, 128), np.float32)
    for h in range(HEADS):
        EgC[h, h*AGENT:(h+1)*AGENT] = SCALE
    HB8 = EgC.copy()   # same pattern masks the gbar broadcast into Eg rows 0..7

    # DIAG: (9, 2, 128, 128) diagonal tap matrices (lhsT layout [K, M])
    dwc9 = dwc_w.reshape(c, 9)
    DIAG = np.zeros((9, 2, 128, 128), np.float32)
    for t in range(9):
        for pt in range(2):
            np.fill_diagonal(DIAG[t, pt], dwc9[pt*128:(pt+1)*128, t])

    BLK = np.zeros((16, 128), np.float32)                            # gbar -> (h,a) expand
    for a in range(16):
        BLK[a, a::16] = 1.0

    NEG9 = np.zeros((128, 18), np.float32)
    for pt in range(2):
        NEG9[:, pt*9:(pt+1)*9] = -dwc9[pt*128:(pt+1)*128, :]

    projb = np.stack([proj_b[:128], proj_b[128:]], axis=1).astype(np.float32)  # (128, 2)
    bvcol = np.stack([bv[:128], bv[128:]], axis=1).astype(np.float32)          # (128, 2)
    dwbcol = np.stack([dwc_b[:128], dwc_b[128:]], axis=1).astype(np.float32)   # (128, 2)

    flags = dict(
        has_qb=bool(np.any(q_b != 0)),
        has_kvb_v=bool(np.any(bv != 0)),
        has_dwcb=bool(np.any(dwc_b != 0)),
        has_projb=True,  # cheap, always fused in the final STT
    )

    params = dict(
        PB=PB.astype(BF), ABt=ABt.astype(BF), MM=MM, Wv=Wv,
        PW=proj_w.astype(np.float32).astype(BF),
        DIAG=DIAG.astype(BF), HM=headmask.astype(BF), EgC=EgC.astype(BF),
        BLK=BLK.astype(BF), HB8=HB8.astype(BF), I16=np.eye(16, dtype=np.float32).astype(BF),
        IDENT=np.eye(128, dtype=np.float32).astype(BF),
        qrows=qrows.astype(BF), NEG9=NEG9, projb=projb, bvcol=bvcol, dwbcol=dwbcol,
    )
    return params, flags


# ----------------------------------------------------------------------------
# device kernel builder
# ----------------------------------------------------------------------------

def _build(flags):
    nc = bacc.Bacc(None, target_bir_lowering=False, debug=False)

    # ---- DRAM I/O ----
    x_in = [nc.dram_tensor(f"x{m+1}", [DIM, N], F32, kind="ExternalInput") for m in range(2)]
    gblk = nc.dram_tensor("gblk", [16, 256], F32, kind="ExternalInput")
    gcols = nc.dram_tensor("gcols", [128, NCH], F32, kind="ExternalInput")
    dPB = nc.dram_tensor("PB", [128, N], BF16, kind="ExternalInput")
    dABt = nc.dram_tensor("ABt", [N, 128], BF16, kind="ExternalInput")
    dMM = nc.dram_tensor("MM", [DIM, 16], F32, kind="ExternalInput")
    dWv = nc.dram_tensor("Wv", [DIM, DIM], F32, kind="ExternalInput")
    dPW = nc.dram_tensor("PW", [DIM, DIM], BF16, kind="ExternalInput")
    dDIAG = nc.dram_tensor("DIAG", [9, 2, 128, 128], BF16, kind="ExternalInput")
    dHM = nc.dram_tensor("HM", [128, DIM], BF16, kind="ExternalInput")
    dEgC = nc.dram_tensor("EgC", [8, 128], BF16, kind="ExternalInput")
    dBLK = nc.dram_tensor("BLK", [16, 128], BF16, kind="ExternalInput")
    dI16 = nc.dram_tensor("I16", [16, 16], BF16, kind="ExternalInput")
    dID = nc.dram_tensor("IDENT", [128, 128], BF16, kind="ExternalInput")
    dqrows = nc.dram_tensor("qrows", [1, 384], BF16, kind="ExternalInput")
    dHB8 = nc.dram_tensor("HB8", [8, 128], BF16, kind="ExternalInput")
    dprojb = nc.dram_tensor("projb", [128, 2], F32, kind="ExternalInput")
    dNEG9 = nc.dram_tensor("NEG9", [128, 18], F32, kind="ExternalInput")
    dbvcol = nc.dram_tensor("bvcol", [128, 2], F32, kind="ExternalInput")
    ddwb = nc.dram_tensor("dwb", [128, 2], F32, kind="ExternalInput")
    o_out = [nc.dram_tensor(f"o{m+1}", [DIM, N], F32, kind="ExternalOutput") for m in range(2)]

    with TileContext(nc) as tc:
        with (
            tc.tile_pool(name="wpool", bufs=1) as wp,          # weights/consts
            tc.tile_pool(name="big", bufs=1) as bigp,          # big per-branch tensors
            tc.tile_pool(name="xpool", bufs=2) as xp,          # input prefetch
            tc.tile_pool(name="small", bufs=3) as sp,          # rotating small tiles
            tc.tile_pool(name="ps_big", bufs=3, space="PSUM") as psb,    # (128,512)
            tc.tile_pool(name="ps_half", bufs=2, space="PSUM") as psh,   # (128,256)
            tc.tile_pool(name="ps_av", bufs=1, space="PSUM") as psav,    # accumulators
            tc.tile_pool(name="ps_sm", bufs=2, space="PSUM") as pssm,    # (128,128)
        ):
            # ---------------- weights & consts into SBUF ----------------
            ident = wp.tile([128, 128], BF16)
            nc.sync.dma_start(ident[:], dID[:])
            i16 = wp.tile([16, 16], BF16)
            nc.sync.dma_start(i16[:], dI16[:])
            blk = wp.tile([16, 128], BF16)
            nc.sync.dma_start(blk[:], dBLK[:])
            hm = wp.tile([128, DIM], BF16)
            nc.sync.dma_start(hm[:], dHM[:])
            pw = wp.tile([128, 2 * DIM], BF16)   # PW as 2 K-half tiles side by side
            nc.sync.dma_start(pw[:, 0:DIM], dPW[0:128, :])
            nc.sync.dma_start(pw[:, DIM:2*DIM], dPW[128:256, :])
            diag = wp.tile([128, 18 * 128], BF16)
            nc.sync.dma_start(
                diag[:].rearrange("p (t m) -> p t m", t=18),
                dDIAG[:].rearrange("t q k m -> k (t q) m"))
            qrows = wp.tile([1, 384], BF16)
            hb8 = wp.tile([8, 128], BF16)
            nc.sync.dma_start(hb8[:], dHB8[:])
            nc.sync.dma_start(qrows[:], dqrows[:])
            neg9 = wp.tile([128, 18], F32)
            nc.sync.dma_start(neg9[:], dNEG9[:])
            projb = wp.tile([128, 2], F32)
            nc.sync.dma_start(projb[:], dprojb[:])
            bvcol = wp.tile([128, 2], F32)
            nc.sync.dma_start(bvcol[:], dbvcol[:])
            dwbcol = wp.tile([128, 2], F32)
            nc.sync.dma_start(dwbcol[:], ddwb[:])

            wv_f = wp.tile([128, 2 * DIM], F32)
            nc.sync.dma_start(wv_f[:, 0:DIM], dWv[0:128, :])
            nc.sync.dma_start(wv_f[:, DIM:2*DIM], dWv[128:256, :])
            wv = wp.tile([128, 2 * DIM], F32R)
            nc.vector.tensor_copy(wv[:], wv_f[:])

            mm_f = wp.tile([128, 2 * 16], F32)
            nc.sync.dma_start(mm_f[:, 0:16], dMM[0:128, :])
            nc.sync.dma_start(mm_f[:, 16:32], dMM[128:256, :])
            mmw = wp.tile([128, 2 * 16], F32R)
            nc.vector.tensor_copy(mmw[:], mm_f[:])

            pb = wp.tile([128, N], BF16)
            nc.sync.dma_start(pb[:], dPB[:])
            abt = wp.tile([128, NCH * 128], BF16)
            nc.sync.dma_start(
                abt[:].rearrange("p (j f) -> p j f", j=NCH),
                dABt[:].rearrange("(j p) f -> p j f", j=NCH))

            # ---------------- gbar & Eg ----------------
            gblk_t = wp.tile([16, 256], F32)
            nc.sync.dma_start(gblk_t[:], gblk[:])
            gsum = wp.tile([16, 1], F32)
            nc.vector.tensor_reduce(gsum[:], gblk_t[:], mybir.AxisListType.X, AL.add)
            gbar_col = wp.tile([16, 1], BF16)
            nc.vector.tensor_scalar(gbar_col[:], gsum[:], 1.0 / 256.0, None, AL.mult)

            ps_g2 = pssm.tile([1, 128], F32, tag="sm")
            nc.tensor.matmul(ps_g2[:], gbar_col[:], blk[:], start=True, stop=True)
            gbar128 = wp.tile([1, 128], BF16)
            nc.scalar.copy(gbar128[:], ps_g2[:])

            eg = wp.tile([16, 128], BF16)
            nc.vector.memset(eg[:], 0.0)
            if flags["has_qb"]:
                nc.sync.dma_start(eg[8:16, :], dEgC[:])
            ones8 = wp.tile([1, 8], BF16)
            nc.vector.memset(ones8[:], 1.0)
            ps_e = pssm.tile([8, 128], F32, tag="sm")
            nc.tensor.matmul(ps_e[:], ones8[:], gbar128[:], start=True, stop=True)
            nc.vector.tensor_tensor(eg[0:8, :], ps_e[:], hb8[:], AL.mult)

            # ---------------- q-path rows ----------------
            u_row = wp.tile([1, 128], BF16)
            nc.vector.tensor_tensor(u_row[:], gbar128[:], qrows[:, 0:128], AL.mult)
            if flags["has_qb"]:
                nc.vector.tensor_tensor(u_row[:], u_row[:], qrows[:, 128:256], AL.add)
                r_row = wp.tile([1, 128], BF16)
                nc.vector.tensor_tensor(r_row[:], gbar128[:], qrows[:, 128:256], AL.mult)
                nc.vector.tensor_tensor(r_row[:], r_row[:], qrows[:, 256:384], AL.add)

            ones1 = wp.tile([1, 128], BF16)
            nc.vector.memset(ones1[:], 1.0)
            ps_u = pssm.tile([128, 128], F32, tag="sm")
            nc.tensor.matmul(ps_u[:], ones1[:], u_row[:], start=True, stop=True)
            u128 = wp.tile([128, 128], BF16)
            nc.scalar.copy(u128[:], ps_u[:])
            if flags["has_qb"]:
                ps_r = pssm.tile([128, 128], F32, tag="sm")
                nc.tensor.matmul(ps_r[:], ones1[:], r_row[:], start=True, stop=True)
                r128 = wp.tile([128, 128], BF16)
                nc.scalar.copy(r128[:], ps_r[:])

            gcols_t = wp.tile([128, NCH], F32)
            nc.sync.dma_start(gcols_t[:], gcols[:])

            # ---------------- q_attn (shared by both branches) ----------------
            qnt = wp.tile([128, N], BF16)        # (h,a) x n, normalized q-attn
            for j in range(NCH):
                lq = sp.tile([128, 128], BF16, tag="lq")
                nc.vector.scalar_tensor_tensor(
                    lq[:], u128[:], gcols_t[:, j:j+1], abt[:, j*128:(j+1)*128],
                    AL.mult, AL.add)
                if flags["has_qb"]:
                    nc.vector.tensor_tensor(lq[:], lq[:], r128[:], AL.add)
                qe = sp.tile([128, 8, 16], BF16, tag="qe")
                nc.scalar.activation(qe[:].rearrange("p a b -> p (a b)"), lq[:], AF.Exp)
                sq = sp.tile([128, 8], F32, tag="sq")
                nc.vector.tensor_reduce(sq[:], qe[:], mybir.AxisListType.X, AL.add)
                rq = sp.tile([128, 8], F32, tag="rq")
                nc.vector.reciprocal(rq[:], sq[:])
                qn = sp.tile([128, 8, 16], BF16, tag="qn")
                nc.vector.tensor_tensor(qn[:], qe[:], rq[:].unsqueeze(2).broadcast_to([128, 8, 16]), AL.mult)
                ps_q = pssm.tile([128, 128], BF16, tag="sm")
                nc.tensor.transpose(ps_q[:], qn[:].rearrange("p a b -> p (a b)"), ident[:])
                nc.scalar.copy(qnt[:, j*128:(j+1)*128], ps_q[:])

            # ---------------- per-branch pipeline ----------------
            for m in range(2):
                xt = [xp.tile([128, N], F32, tag=f"x{pt}", name=f"xt{pt}") for pt in range(2)]
                nc.sync.dma_start(xt[0][:], x_in[m][0:128, :])
                nc.sync.dma_start(xt[1][:], x_in[m][128:256, :])
                # vT (c-major value tensor) + kw streams, from fp32r-rounded x chunks
                vt = [bigp.tile([128, N], BF16, tag=f"vt{pt}", name=f"vt{pt}") for pt in range(2)]
                kwf = bigp.tile([16, N], BF16, tag="kwf")
                for t in range(NT):
                    xrc = [sp.tile([128, NTW], F32R, tag=f"xrc{kh}", name=f"xrc{kh}", bufs=2) for kh in range(2)]
                    for kh in range(2):
                        nc.vector.tensor_copy(xrc[kh][:], xt[kh][:, t*NTW:(t+1)*NTW])
                    for pt in range(2):
                        ps_v = psb.tile([128, NTW], F32, tag="big")
                        for kh in range(2):
                            nc.tensor.matmul(
                                ps_v[:], wv[:, kh*DIM + pt*128: kh*DIM + (pt+1)*128],
                                xrc[kh][:],
                                start=(kh == 0), stop=(kh == 1))
                        sl = vt[pt][:, t*NTW:(t+1)*NTW]
                        if flags["has_kvb_v"]:
                            nc.vector.tensor_scalar(sl, ps_v[:], bvcol[:, pt:pt+1], None, AL.add)
                        elif (pt + t) % 2 == 0:
                            nc.scalar.copy(sl, ps_v[:])
                        else:
                            nc.vector.tensor_copy(sl, ps_v[:])
                    ps_k = psh.tile([16, NTW], F32, tag="half")
                    for kh in range(2):
                        nc.tensor.matmul(
                            ps_k[:], mmw[:, kh*16:(kh+1)*16],
                            xrc[kh][:],
                            start=(kh == 0), stop=(kh == 1))
                    nc.vector.tensor_copy(kwf[:, t*NTW:(t+1)*NTW], ps_k[:])

                # attn logits -> exp -> row sums
                attn = bigp.tile([128, N], BF16, tag="attn")
                s1p = bigp.tile([128, NT], F32, tag="s1p")
                for t in range(NT):
                    ps_l = psb.tile([128, NTW], F32, tag="big")
                    nc.tensor.matmul(ps_l[:], eg[:], kwf[:, t*NTW:(t+1)*NTW],
                                     start=True, stop=True)
                    tmp = sp.tile([128, NTW], F32, tag="ltmp")
                    nc.vector.tensor_tensor(tmp[:], ps_l[:], pb[:, t*NTW:(t+1)*NTW], AL.add)
                    nc.scalar.activation(attn[:, t*NTW:(t+1)*NTW], tmp[:], AF.Exp,
                                         accum_out=s1p[:, t:t+1])
                s1 = bigp.tile([128, 1], F32, tag="s1")
                nc.vector.tensor_reduce(s1[:], s1p[:], mybir.AxisListType.X, AL.add)
                rs1 = bigp.tile([128, 1], F32, tag="rs1")
                nc.vector.reciprocal(rs1[:], s1[:])

                # attn transpose + v row tiles + AVfull accumulation
                ps_av = psav.tile([128, DIM], F32, tag="av")
                at_t = bigp.tile([128, N], BF16, tag="attnT")
                for k in range(NCH):
                    ps_t = pssm.tile([128, 128], BF16, tag="sm")
                    nc.tensor.transpose(ps_t[:], attn[:, k*128:(k+1)*128], ident[:])
                    atk = at_t[:, k*128:(k+1)*128]
                    if k % 2 == 0:
                        nc.scalar.copy(atk, ps_t[:])
                    else:
                        nc.vector.tensor_copy(atk, ps_t[:])

                    ps_vr = psh.tile([128, DIM], BF16, tag="half")
                    for pt in range(2):
                        nc.tensor.transpose(ps_vr[:, pt*128:(pt+1)*128],
                                            vt[pt][:, k*128:(k+1)*128], ident[:])
                    vrk = sp.tile([128, DIM], BF16, tag="vrow_sb")
                    if k % 2 == 0:
                        nc.vector.tensor_copy(vrk[:], ps_vr[:])
                    else:
                        nc.scalar.copy(vrk[:], ps_vr[:])
                    nc.tensor.matmul(ps_av[:], at_t[:, k*128:(k+1)*128], vrk[:],
                                     start=(k == 0), stop=(k == NCH - 1))

                avsel = bigp.tile([128, DIM], BF16, tag="avsel")
                nc.vector.scalar_tensor_tensor(avsel[:], ps_av[:], rs1[:], hm[:],
                                               AL.mult, AL.mult)

                # N1T + depthwise conv accumulated in one PSUM, then proj
                pre = [bigp.tile([128, N], BF16, tag=f"pre{pt}", name=f"pre{pt}") for pt in range(2)]
                diag3 = diag[:].rearrange("p (t m) -> p t m", t=18)
                for pt in range(2):
                    for t in range(NT):
                        ps_n = psb.tile([128, NTW], F32, tag="big")
                        # 9 shifted diagonal taps, flat contiguous ranges;
                        # center tap (full range) opens the accumulation group
                        taps = []
                        tap = 0
                        for dy in (-1, 0, 1):
                            a0, b0 = max(0, -dy), H - max(0, dy)
                            for dx in (-1, 0, 1):
                                s = dy * W + dx
                                lo = max(t * NTW, a0 * W, -s)
                                hi = min((t + 1) * NTW, b0 * W, N - s)
                                taps.append((tap, lo, hi, s))
                                tap += 1
                        taps.sort(key=lambda q: q[0] != 4)   # center first
                        for tap, lo, hi, s in taps:
                            nc.tensor.matmul(
                                ps_n[:, lo - t*NTW:hi - t*NTW],
                                diag3[:, tap*2 + pt, :],
                                vt[pt][:, lo + s:hi + s],
                                start=(tap == 4), stop=False,
                                skip_group_check=True)
                        nc.tensor.matmul(ps_n[:],
                                         avsel[:, pt*128:(pt+1)*128],
                                         qnt[:, t*NTW:(t+1)*NTW],
                                         start=False, stop=True,
                                         skip_group_check=True)
                        sl = pre[pt][:, t*NTW:(t+1)*NTW]
                        if flags["has_dwcb"]:
                            nc.vector.tensor_scalar(sl, ps_n[:],
                                                    dwbcol[:, pt:pt+1], None, AL.add)
                        else:
                            nc.scalar.copy(sl, ps_n[:])
                    # x-wraparound border corrections for the 6 dx!=0 taps
                    pre3 = pre[pt][:].rearrange("p (y x) -> p y x", y=H)
                    vt3 = vt[pt][:].rearrange("p (y x) -> p y x", y=H)
                    for dy, k, ysrc, ydst in (
                        (-1, 2, (1, 64), (1, 64)),    # (dy=-1,dx=+1): v[y,0] -> pre[y,63]
                        (0, 5, (1, 64), (0, 63)),     # (dy=0, dx=+1): v[y+1,0] -> pre[y,63]
                        (1, 8, (2, 64), (0, 62)),     # (dy=+1,dx=+1): v[y+2,0] -> pre[y,63]
                    ):
                        nc.vector.scalar_tensor_tensor(
                            pre3[:, ydst[0]:ydst[1], 63:64],
                            vt3[:, ysrc[0]:ysrc[1], 0:1],
                            neg9[:, pt*9 + k:pt*9 + k + 1],
                            pre3[:, ydst[0]:ydst[1], 63:64],
                            AL.mult, AL.add)
                    for dy, k, ysrc, ydst in (
                        (-1, 0, (0, 62), (2, 64)),    # (dy=-1,dx=-1): v[y-2,63] -> pre[y,0]
                        (0, 3, (0, 63), (1, 64)),     # (dy=0, dx=-1): v[y-1,63] -> pre[y,0]
                        (1, 6, (0, 63), (0, 63)),     # (dy=+1,dx=-1): v[y,63] -> pre[y,0]
                    ):
                        nc.vector.scalar_tensor_tensor(
                            pre3[:, ydst[0]:ydst[1], 0:1],
                            vt3[:, ysrc[0]:ysrc[1], 63:64],
                            neg9[:, pt*9 + k:pt*9 + k + 1],
                            pre3[:, ydst[0]:ydst[1], 0:1],
                            AL.mult, AL.add)

                for mt in range(2):
                    for t in range(NT):
                        ps_o = psb.tile([128, NTW], F32, tag="big")
                        for kh in range(2):
                            nc.tensor.matmul(
                                ps_o[:], pw[:, kh*DIM + mt*128: kh*DIM + (mt+1)*128],
                                pre[kh][:, t*NTW:(t+1)*NTW],
                                start=(kh == 0), stop=(kh == 1))
                        ot = sp.tile([128, NTW], F32, tag="ot", bufs=3)
                        nc.vector.scalar_tensor_tensor(
                            ot[:], ps_o[:], projb[:, mt:mt+1],
                            xt[mt][:, t*NTW:(t+1)*NTW], AL.add, AL.add)
                        nc.sync.dma_start(o_out[m][mt*128:(mt+1)*128, t*NTW:(t+1)*NTW], ot[:])

    nc.compile()
    return nc


# ----------------------------------------------------------------------------
# public entry point
# ----------------------------------------------------------------------------

_CACHE = {}


def kernel(**inputs):
    inputs = {k: np.asarray(v) for k, v in inputs.items()}
    params, flags = _host_precompute(
        **{k: inputs[k] for k in
           ("kv_w", "kv_b", "q_w", "q_b", "proj_w", "proj_b", "dwc_w", "dwc_b",
            "an_bias", "na_bias", "ah_bias", "aw_bias", "ha_bias", "wa_bias")})

    key = tuple(sorted(flags.items()))
    if key not in _CACHE:
        _CACHE[key] = _build(flags)
    nc = _CACHE[key]

    input1, input2, guidmap = inputs["input1"], inputs["input2"], inputs["guidmap"]
    shared = {
        "PB": params["PB"], "ABt": params["ABt"], "MM": params["MM"],
        "Wv": params["Wv"], "PW": params["PW"], "DIAG": params["DIAG"],
        "HM": params["HM"], "EgC": params["EgC"], "BLK": params["BLK"],
        "I16": params["I16"], "IDENT": params["IDENT"], "qrows": params["qrows"], "HB8": params["HB8"],
        "projb": params["projb"], "NEG9": params["NEG9"], "bvcol": params["bvcol"], "dwb": params["dwbcol"],
    }
    in_maps = []
    for b in range(B):
        g = guidmap[b].reshape(N).astype(np.float32)
        gimg = g.reshape(H, W)
        gblk = gimg.reshape(PS, AGENT, PS, AGENT).transpose(0, 2, 1, 3).reshape(AGENT, 256)
        gcols = g.reshape(NCH, 128).T.copy()
        in_maps.append({
            "x1": np.ascontiguousarray(input1[b].reshape(DIM, N)),
            "x2": np.ascontiguousarray(input2[b].reshape(DIM, N)),
            "gblk": np.ascontiguousarray(gblk.astype(np.float32)),
            "gcols": np.ascontiguousarray(gcols.astype(np.float32)),
            **shared,
        })

    res = run_bass_kernel_spmd(nc, in_maps, core_ids=list(range(B)))
    o1 = np.stack([res.results[b]["o1"].reshape(DIM, H, W) for b in range(B)])
    o2 = np.stack([res.results[b]["o2"].reshape(DIM, H, W) for b in range(B)])
    return o1.astype(np.float32), o2.astype(np.float32)
